# revision 13
# baseline (speedup 1.0000x reference)
"""Trainium2 Bass kernel for nn_Context_Erased_Attention_Advanced.

Computation (per batch row b, K=6 objects):
  joint  = relu([v_bk, q_b] @ W1.T + b1)          # [K, NHID]
  logit  = joint @ Wl.T + bl                      # [K, 1]
  single_att = softmax_k(logit)                   # output 1: [B, K, 1]
  w_ctx  = logit * mask
  neigh  = sum_k(w_ctx) - w_ctx
  y      = sigmoid(SE(neigh))    (SE = FC(1->128) relu FC(128->1))
  s      = neigh * (1 - y) + logit
  ctx    = s * Wf + bf                            # output 2: [B*K, 1, VDIM]

Strategy: pure data parallel over 8 NeuronCores (batch shards of 1024).
On-core: split W1 into v-part / q-part; q matmul done once per 128-row
b-tile, v matmul per (b-tile, k).  Activations must land contraction-dim
(feature) on SBUF partitions, so v/q row tiles are transposed on the PE
(identity matmul).  Main matmuls run as float32r (full PE rate at free
size 512).  The SE MLP collapses exactly to a piecewise-linear scalar
function: SE(x) = c1*relu(x) + c2*x with host-computed c1, c2.
"""

import numpy as np
from contextlib import ExitStack

import concourse.bass as bass
import concourse.tile as tile
from concourse import mybir

F32 = mybir.dt.float32
F32R = mybir.dt.float32r
AX = mybir.AxisListType
OP = mybir.AluOpType
AF = mybir.ActivationFunctionType

B, K, VDIM, QDIM, NHID = 8192, 6, 2048, 1024, 1024
N_CORES = 8
B_LOC = B // N_CORES  # 1024 rows per core
BT = 128              # rows per b-tile (SBUF partition dim)
NV = VDIM // 128      # 16 contraction chunks for the v matmul
NQ = QDIM // 128      # 8 contraction chunks for the q matmul
NB = NHID // 512      # 2 PSUM column banks for joint


def emit(tc, ap, b_loc):
    nc = tc.nc
    n_bt = b_loc // BT
    with ExitStack() as ctx:
        const = ctx.enter_context(tc.tile_pool(name="const", bufs=1))
        vin = ctx.enter_context(tc.tile_pool(name="vin", bufs=2))
        vtp = ctx.enter_context(tc.tile_pool(name="vtp", bufs=2))
        qin = ctx.enter_context(tc.tile_pool(name="qin", bufs=1))
        qtp = ctx.enter_context(tc.tile_pool(name="qtp", bufs=1))
        prep = ctx.enter_context(tc.tile_pool(name="prep", bufs=1))
        hqs = ctx.enter_context(tc.tile_pool(name="hqs", bufs=2))
        outp = ctx.enter_context(tc.tile_pool(name="outp", bufs=2))
        smallp = ctx.enter_context(tc.tile_pool(name="smallp", bufs=2))
        tpsum = ctx.enter_context(tc.tile_pool(name="tpsum", bufs=3, space="PSUM"))
        jpsum = ctx.enter_context(tc.tile_pool(name="jpsum", bufs=2, space="PSUM"))

        # --- constants into SBUF (once); small ones first on the fast HWDGE
        # queues so the first b-tile's transposes are not stuck behind the
        # 12.6MB of W1 weights (w1qt via ACT HWDGE, w1vt via gpsimd SWDGE) ---
        ident = const.tile([128, 128], F32R)
        nc.sync.dma_start(ident[:], ap["ident"][:])
        ones = const.tile([1, 128], F32R)
        nc.sync.dma_start(ones[:], ap["ones_row"][:])
        scal = const.tile([128, 8], F32)
        nc.sync.dma_start(scal[:], ap["scal"][:])
        wl = const.tile([128, NHID], F32)
        nc.sync.dma_start(wl[:], ap["wl_b"][:])
        wf = const.tile([128, VDIM], F32)
        nc.sync.dma_start(wf[:], ap["wf_b"][:])
        bfb = const.tile([128, VDIM], F32)
        nc.sync.dma_start(bfb[:], ap["bf_b"][:])
        # per-chunk weight tiles: a matmul on chunk c then depends only on
        # chunk c's DMA, so the PE streams at DMA pace instead of waiting
        # for the whole 8MB. First v-chunks ride the fast sync HWDGE queue.
        w1qt = [const.tile([128, NHID], F32R, tag=f"w1qt{c}", name=f"w1qt{c}")
                for c in range(NQ + 1)]
        for c in range(NQ + 1):
            nc.scalar.dma_start(w1qt[c][:], ap["w1qtb"][c])
        w1vt = [const.tile([128, NHID], F32R, tag=f"w1vt{c}", name=f"w1vt{c}")
                for c in range(NV)]
        for c in range(NV):
            if c < 4:
                nc.sync.dma_start(w1vt[c][:], ap["w1vt"][c])
            else:
                for p0 in (0, 64):
                    nc.gpsimd.dma_start(w1vt[c][p0:p0 + 64, :],
                                        ap["w1vt"][c, p0:p0 + 64, :])
        c1_ap = scal[:, 0:1]
        c2_ap = scal[:, 1:2]
        bl_ap = scal[:, 2:3]

        v3 = ap["v"]
        ctx3 = ap["ctx"].rearrange("(b k) j -> b k j", k=K)

        def transpose_128(dst, dst_g, src, g, copy_eng):
            """Transpose 4 [128,128] chunks of src into one PSUM bank, then
            one batched copy to dst[:, 4g:4g+4, :]."""
            pt = tpsum.tile([128, 4, 128], F32R, tag="tp")
            for t in range(4):
                c = g * 4 + t
                nc.tensor.transpose(pt[:, t, :],
                                    src[:, c * 128:(c + 1) * 128], ident[:])
            if copy_eng == "v":
                nc.vector.tensor_copy(out=dst[:, dst_g * 4:(dst_g + 1) * 4, :],
                                      in_=pt[:])
            else:
                nc.scalar.copy(dst[:, dst_g * 4:(dst_g + 1) * 4, :], pt[:])

        def load_vT(b0, k):
            vn = vin.tile([128, VDIM], F32R)
            nc.sync.dma_start(vn[:], v3[b0:b0 + BT, k, :])
            vt = vtp.tile([128, NV, 128], F32R)
            for g in range(NV // 4):
                transpose_128(vt, g, vn, g, "v" if g % 3 == 2 else "s")
            return vt

        def q_phase(b0):
            qn = qin.tile([128, QDIM], F32R)
            nc.sync.dma_start(qn[:], ap["q"][b0:b0 + BT, :])
            qt = qtp.tile([128, NQ, 128], F32R)
            for g in range(NQ // 4):
                transpose_128(qt, g, qn, g, "s" if g % 2 == 0 else "v")
            return qt

        def q_matmul(qt):
            """hq = q @ W1q.T + b1 into a PSUM slot, then to SBUF."""
            hq = jpsum.tile([128, NHID], F32, tag="j")
            for c in range(NQ):
                for j in range(NB):
                    nc.tensor.matmul(
                        hq[:, j * 512:(j + 1) * 512],
                        lhsT=qt[:, c, :],
                        rhs=w1qt[c][:, j * 512:(j + 1) * 512],
                        start=(c == 0), stop=False)
            for j in range(NB):
                # + b1 as a rank-1 update (ones x b1-row, packed as chunk NQ)
                nc.tensor.matmul(
                    hq[:, j * 512:(j + 1) * 512],
                    lhsT=ones[:],
                    rhs=w1qt[NQ][0:1, j * 512:(j + 1) * 512],
                    start=False, stop=True)
            hq_sb = hqs.tile([128, NHID], F32)
            nc.scalar.copy(hq_sb[:], hq[:])
            return hq_sb

        qt_cur = q_phase(0)
        for bt in range(n_bt):
            b0 = bt * BT
            vts = {0: load_vT(b0, 0)}
            # bt 0: defer the q matmul until after k=0's matmuls so the PE
            # does DMA-paced work while the weights stream in
            hq_sb = q_matmul(qt_cur) if bt > 0 else None

            s6 = smallp.tile([128, 16, K], F32, tag="s6")
            s1 = smallp.tile([128, 8], F32, tag="s1")
            logits = s6[:, 0, :]
            mk = s6[:, 1, :]
            nc.sync.dma_start(mk, ap["mask"][b0:b0 + BT, :])

            # ---- per-object joint matmul + logit ----
            for k in range(K):
                if k + 1 < K:
                    # transposes for the next object ahead of this one's
                    # matmuls so the PE never waits on the copy engines
                    vts[k + 1] = load_vT(b0, k + 1)
                if k == 2 and bt + 1 < n_bt:
                    # prefetch the next b-tile's q transposes mid-loop
                    qt_cur = q_phase((bt + 1) * BT)
                vt = vts.pop(k)
                jp = jpsum.tile([128, NHID], F32, tag="j")
                for c in range(NV):
                    for j in range(NB):
                        nc.tensor.matmul(
                            jp[:, j * 512:(j + 1) * 512],
                            lhsT=vt[:, c, :],
                            rhs=w1vt[c][:, j * 512:(j + 1) * 512],
                            start=(c == 0),
                            stop=(c == NV - 1),
                        )
                if bt == 0 and k == 0:
                    hq_sb = q_matmul(qt_cur)
                pre = prep.tile([128, NHID], F32)
                nc.vector.tensor_add(pre[:], jp[:], hq_sb[:])
                # logit_k = sum_h(relu(pre) * Wl) in one fused DVE pass
                nc.vector.scalar_tensor_tensor(
                    jp[:], pre[:], 0.0, wl[:],
                    op0=OP.max, op1=OP.mult,
                    accum_out=s6[:, 0, k:k + 1],
                )
            # + bl
            nc.vector.tensor_scalar_add(logits, logits, bl_ap)

            # ---- softmax over k ----
            nc.vector.tensor_reduce(s1[:, 0:1], logits, axis=AX.X, op=OP.max,
                                    negate=True)
            e_sm = s6[:, 2, :]
            nc.scalar.activation(e_sm, logits, AF.Exp, bias=s1[:, 0:1])
            nc.vector.tensor_reduce(s1[:, 1:2], e_sm, axis=AX.X, op=OP.add)
            nc.vector.reciprocal(s1[:, 2:3], s1[:, 1:2])
            att = s6[:, 3, :]
            nc.vector.tensor_scalar_mul(att, e_sm, s1[:, 2:3])
            nc.scalar.dma_start(ap["att"][b0:b0 + BT, :], att)

            # ---- context-erased scalar chain ----
            w_ctx = s6[:, 4, :]
            nc.vector.tensor_mul(w_ctx, logits, mk)
            nc.vector.tensor_reduce(s1[:, 3:4], w_ctx, axis=AX.X, op=OP.add)
            x = s6[:, 5, :]
            # x = (w_ctx - rowsum) * -1
            nc.vector.tensor_scalar(x, w_ctx, s1[:, 3:4], -1.0,
                                    op0=OP.subtract, op1=OP.mult)
            t1 = s6[:, 6, :]
            # t1 = relu(x) * c1
            nc.vector.tensor_scalar(t1, x, 0.0, c1_ap, op0=OP.max, op1=OP.mult)
            inner = s6[:, 7, :]
            # inner = x * c2 + t1
            nc.vector.scalar_tensor_tensor(inner, x, c2_ap, t1,
                                           op0=OP.mult, op1=OP.add)
            e_se = s6[:, 8, :]
            nc.scalar.activation(e_se, inner, AF.Exp, scale=-1.0)
            d = s6[:, 9, :]
            nc.vector.tensor_scalar_add(d, e_se, 1.0)
            r = s6[:, 10, :]
            nc.vector.reciprocal(r, d)
            g1 = s6[:, 11, :]
            nc.vector.tensor_mul(g1, x, e_se)
            gated = s6[:, 12, :]
            nc.vector.tensor_mul(gated, g1, r)
            s = s6[:, 13, :]
            nc.vector.tensor_add(s, gated, logits)

            # ---- ctx output: s * Wf + bf in one fused DVE pass per k ----
            for k in range(K):
                o = outp.tile([128, VDIM], F32)
                nc.vector.scalar_tensor_tensor(o[:], wf[:], s6[:, 13, k:k + 1],
                                               bfb[:], op0=OP.mult, op1=OP.add)
                nc.scalar.dma_start(ctx3[b0:b0 + BT, k, :], o[:])


def split_waits(nc, max_waits=1):
    """Walrus codegen rejects >1 sync wait on one instruction; split extras
    into preceding NoOps on the same engine."""
    for f in nc.m.functions:
        for bb in f.blocks:
            new_list = []
            changed = False
            for inst in bb.instructions:
                si = inst.sync_info
                if si is not None and len(si.on_wait) > max_waits:
                    waits = list(si.on_wait)
                    k = 0
                    while len(waits) > max_waits:
                        chunk, waits = waits[:max_waits], waits[max_waits:]
                        n = mybir.InstNoOp(name=f"{inst.name}-wsplit{k}",
                                           ins=[], outs=[])
                        n.engine = inst.engine
                        n.sync_info = mybir.SyncInfo(on_wait=chunk, on_update=[])
                        new_list.append(n)
                        k += 1
                    inst.sync_info = mybir.SyncInfo(on_wait=waits,
                                                    on_update=list(si.on_update))
                    changed = True
                new_list.append(inst)
            if changed:
                bb.instructions = new_list


def build(b_loc=B_LOC):
    nc = bass.Bass("TRN2", target_bir_lowering=False, debug=False)
    d = {}
    def din(name, shape, dt=F32):
        d[name] = nc.dram_tensor(name, shape, dt, kind="ExternalInput").ap()
    din("v", [b_loc, K, VDIM], F32R)
    din("q", [b_loc, QDIM], F32R)
    din("mask", [b_loc, K])
    din("w1vt", [NV, 128, NHID], F32R)
    din("w1qtb", [NQ + 1, 128, NHID], F32R)
    din("wl_b", [128, NHID])
    din("wf_b", [128, VDIM])
    din("bf_b", [128, VDIM])
    din("ident", [128, 128], F32R)
    din("ones_row", [1, 128], F32R)
    din("scal", [128, 8])
    d["att"] = nc.dram_tensor("att", [b_loc, K], F32, kind="ExternalOutput").ap()
    d["ctx"] = nc.dram_tensor("ctx", [b_loc * K, VDIM], F32,
                              kind="ExternalOutput").ap()
    with tile.TileContext(nc) as tc:
        emit(tc, d, b_loc)
    return nc


def host_consts(W1, b1, Wl, bl, Wse1, Wse2, Wf, bf):
    W1 = np.asarray(W1, np.float32)
    w1vt = np.ascontiguousarray(W1[:, :VDIM].T).reshape(NV, 128, NHID)
    w1qtb = np.zeros((NQ + 1, 128, NHID), np.float32)
    w1qtb[:NQ] = np.ascontiguousarray(W1[:, VDIM:].T).reshape(NQ, 128, NHID)
    w1qtb[NQ, 0, :] = np.asarray(b1, np.float32)
    wl_b = np.tile(np.asarray(Wl, np.float32).reshape(1, NHID), (128, 1))
    wf_b = np.tile(np.asarray(Wf, np.float32).reshape(1, VDIM), (128, 1))
    bf_b = np.tile(np.asarray(bf, np.float32).reshape(1, VDIM), (128, 1))
    a = np.asarray(Wse1, np.float64).reshape(-1)
    c = np.asarray(Wse2, np.float64).reshape(-1)
    P = float(np.sum(np.where(a > 0, a * c, 0.0)))
    Nn = float(np.sum(np.where(a < 0, a * c, 0.0)))
    scal = np.zeros((128, 8), np.float32)
    scal[:, 0] = P - Nn          # c1
    scal[:, 1] = Nn              # c2
    scal[:, 2] = float(np.asarray(bl).reshape(-1)[0])
    return {
        "w1vt": np.ascontiguousarray(w1vt),
        "w1qtb": w1qtb,
        "wl_b": np.ascontiguousarray(wl_b),
        "wf_b": np.ascontiguousarray(wf_b),
        "bf_b": np.ascontiguousarray(bf_b),
        "ident": np.eye(128, dtype=np.float32),
        "ones_row": np.ones((1, 128), np.float32),
        "scal": scal,
    }


_NC_CACHE = {}


def _get_nc(b_loc=B_LOC):
    if b_loc not in _NC_CACHE:
        nc = build(b_loc)
        split_waits(nc)
        _NC_CACHE[b_loc] = nc
    return _NC_CACHE[b_loc]


def run(inputs, trace=False):
    from concourse.bass_utils import run_bass_kernel_spmd

    v = np.ascontiguousarray(np.asarray(inputs["v"], np.float32))
    q = np.ascontiguousarray(np.asarray(inputs["q"], np.float32))
    mask = np.ascontiguousarray(np.asarray(inputs["mask"], np.float32))
    consts = host_consts(inputs["W1"], inputs["b1"], inputs["Wl"], inputs["bl"],
                         inputs["Wse1"], inputs["Wse2"], inputs["Wf"],
                         inputs["bf"])
    nc = _get_nc()
    in_maps = []
    for i in range(N_CORES):
        s = slice(i * B_LOC, (i + 1) * B_LOC)
        in_maps.append({
            "v": v[s], "q": q[s], "mask": mask[s], **consts,
        })
    res = run_bass_kernel_spmd(nc, in_maps, core_ids=list(range(N_CORES)),
                               trace=trace)
    att = np.concatenate([r["att"] for r in res.results], axis=0)
    ctxo = np.concatenate([r["ctx"] for r in res.results], axis=0)
    single_att = att.reshape(B, K, 1)
    context_erased_att = ctxo.reshape(B * K, 1, VDIM)
    return (single_att, context_erased_att), res


def kernel(**inputs):
    out, _ = run(inputs, trace=False)
    return out


# revision 14
# speedup vs baseline: 1.0845x; 1.0845x over previous
"""Trainium2 Bass kernel for nn_Context_Erased_Attention_Advanced.

Computation (per batch row b, K=6 objects):
  joint  = relu([v_bk, q_b] @ W1.T + b1)          # [K, NHID]
  logit  = joint @ Wl.T + bl                      # [K, 1]
  single_att = softmax_k(logit)                   # output 1: [B, K, 1]
  w_ctx  = logit * mask
  neigh  = sum_k(w_ctx) - w_ctx
  y      = sigmoid(SE(neigh))    (SE = FC(1->128) relu FC(128->1))
  s      = neigh * (1 - y) + logit
  ctx    = s * Wf + bf                            # output 2: [B*K, 1, VDIM]

Strategy: pure data parallel over 8 NeuronCores (batch shards of 1024).
On-core: split W1 into v-part / q-part; q matmul done once per 128-row
b-tile, v matmul per (b-tile, k).  Activations must land contraction-dim
(feature) on SBUF partitions, so v/q row tiles are transposed on the PE
(identity matmul).  Main matmuls run as float32r (full PE rate at free
size 512).  The SE MLP collapses exactly to a piecewise-linear scalar
function: SE(x) = c1*relu(x) + c2*x with host-computed c1, c2.
"""

import numpy as np
from contextlib import ExitStack

import concourse.bass as bass
import concourse.tile as tile
from concourse import mybir

F32 = mybir.dt.float32
F32R = mybir.dt.float32r
AX = mybir.AxisListType
OP = mybir.AluOpType
AF = mybir.ActivationFunctionType

B, K, VDIM, QDIM, NHID = 8192, 6, 2048, 1024, 1024
N_CORES = 8
B_LOC = B // N_CORES  # 1024 rows per core
BT = 128              # rows per b-tile (SBUF partition dim)
NV = VDIM // 128      # 16 contraction chunks for the v matmul
NQ = QDIM // 128      # 8 contraction chunks for the q matmul
NB = NHID // 512      # 2 PSUM column banks for joint


def emit(tc, ap, b_loc):
    nc = tc.nc
    n_bt = b_loc // BT
    with ExitStack() as ctx:
        const = ctx.enter_context(tc.tile_pool(name="const", bufs=1))
        vin = ctx.enter_context(tc.tile_pool(name="vin", bufs=2))
        vtp = ctx.enter_context(tc.tile_pool(name="vtp", bufs=2))
        qin = ctx.enter_context(tc.tile_pool(name="qin", bufs=1))
        qtp = ctx.enter_context(tc.tile_pool(name="qtp", bufs=1))
        prep = ctx.enter_context(tc.tile_pool(name="prep", bufs=1))
        hqs = ctx.enter_context(tc.tile_pool(name="hqs", bufs=2))
        outp = ctx.enter_context(tc.tile_pool(name="outp", bufs=2))
        smallp = ctx.enter_context(tc.tile_pool(name="smallp", bufs=2))
        tpsum = ctx.enter_context(tc.tile_pool(name="tpsum", bufs=3, space="PSUM"))
        jpsum = ctx.enter_context(tc.tile_pool(name="jpsum", bufs=2, space="PSUM"))

        # --- constants into SBUF (once); small ones first on the fast HWDGE
        # queues so the first b-tile's transposes are not stuck behind the
        # 12.6MB of W1 weights (w1qt via ACT HWDGE, w1vt via gpsimd SWDGE) ---
        ident = const.tile([128, 128], F32R)
        nc.sync.dma_start(ident[:], ap["ident"][:])
        ones = const.tile([1, 128], F32R)
        nc.sync.dma_start(ones[:], ap["ones_row"][:])
        scal = const.tile([128, 8], F32)
        nc.sync.dma_start(scal[:], ap["scal"][:])
        wl = const.tile([128, NHID], F32)
        nc.sync.dma_start(wl[:], ap["wl_b"][:])
        wf = const.tile([128, VDIM], F32)
        nc.sync.dma_start(wf[:], ap["wf_b"][:])
        bfb = const.tile([128, VDIM], F32)
        nc.sync.dma_start(bfb[:], ap["bf_b"][:])
        # per-chunk weight tiles: a matmul on chunk c then depends only on
        # chunk c's DMA, so the PE streams at DMA pace instead of waiting
        # for the whole 8MB. First v-chunks ride the fast sync HWDGE queue.
        w1qt = [const.tile([128, NHID], F32R, tag=f"w1qt{c}", name=f"w1qt{c}")
                for c in range(NQ + 1)]

        w1vt = [const.tile([128, NHID], F32R, tag=f"w1vt{c}", name=f"w1vt{c}")
                for c in range(NV)]
        # even v-chunks stream on the scalar HWDGE queue (16-way-split per
        # DMA, ~1.4us each), odd chunks in 32KB-pieces across the SWDGE
        # queues; q-weights follow the evens.  The sync queue carries only
        # activation data, so the first b-tile is never stuck behind weights.
        for c in range(0, NV, 2):
            nc.scalar.dma_start(w1vt[c][:], ap["w1vt"][c])
        for c in range(NQ + 1):
            nc.scalar.dma_start(w1qt[c][:], ap["w1qtb"][c])
        for c in range(1, NV, 2):
            for p0 in range(0, 128, 32):
                nc.gpsimd.dma_start(w1vt[c][p0:p0 + 32, :],
                                    ap["w1vt"][c, p0:p0 + 32, :])
        c1_ap = scal[:, 0:1]
        c2_ap = scal[:, 1:2]
        bl_ap = scal[:, 2:3]

        v3 = ap["v"]
        ctx3 = ap["ctx"].rearrange("(b k) j -> b k j", k=K)

        def transpose_128(dst, dst_g, src, g, copy_eng):
            """Transpose 4 [128,128] chunks of src into one PSUM bank, then
            one batched copy to dst[:, 4g:4g+4, :]."""
            pt = tpsum.tile([128, 4, 128], F32R, tag="tp")
            for t in range(4):
                c = g * 4 + t
                nc.tensor.transpose(pt[:, t, :],
                                    src[:, c * 128:(c + 1) * 128], ident[:])
            if copy_eng == "v":
                nc.vector.tensor_copy(out=dst[:, dst_g * 4:(dst_g + 1) * 4, :],
                                      in_=pt[:])
            else:
                nc.scalar.copy(dst[:, dst_g * 4:(dst_g + 1) * 4, :], pt[:])

        def load_vT(b0, k):
            vn = vin.tile([128, VDIM], F32R)
            nc.sync.dma_start(vn[:], v3[b0:b0 + BT, k, :])
            vt = vtp.tile([128, NV, 128], F32R)
            for g in range(NV // 4):
                transpose_128(vt, g, vn, g, "v" if g % 3 == 2 else "s")
            return vt

        def q_phase(b0):
            qn = qin.tile([128, QDIM], F32R)
            nc.sync.dma_start(qn[:], ap["q"][b0:b0 + BT, :])
            qt = qtp.tile([128, NQ, 128], F32R)
            for g in range(NQ // 4):
                transpose_128(qt, g, qn, g, "s" if g % 2 == 0 else "v")
            return qt

        def q_matmul(qt):
            """hq = q @ W1q.T + b1 into a PSUM slot, then to SBUF."""
            hq = jpsum.tile([128, NHID], F32, tag="j")
            for c in range(NQ):
                for j in range(NB):
                    nc.tensor.matmul(
                        hq[:, j * 512:(j + 1) * 512],
                        lhsT=qt[:, c, :],
                        rhs=w1qt[c][:, j * 512:(j + 1) * 512],
                        start=(c == 0), stop=False)
            for j in range(NB):
                # + b1 as a rank-1 update (ones x b1-row, packed as chunk NQ)
                nc.tensor.matmul(
                    hq[:, j * 512:(j + 1) * 512],
                    lhsT=ones[:],
                    rhs=w1qt[NQ][0:1, j * 512:(j + 1) * 512],
                    start=False, stop=True)
            hq_sb = hqs.tile([128, NHID], F32)
            nc.scalar.copy(hq_sb[:], hq[:])
            return hq_sb

        qt_cur = q_phase(0)
        for bt in range(n_bt):
            b0 = bt * BT
            vts = {0: load_vT(b0, 0)}
            # q matmul deferred until after k=0's matmuls: the first vMM of
            # each b-tile then only needs the earliest-freed PSUM slot
            hq_sb = None

            s6 = smallp.tile([128, 16, K], F32, tag="s6")
            s1 = smallp.tile([128, 8], F32, tag="s1")
            logits = s6[:, 0, :]
            mk = s6[:, 1, :]
            nc.sync.dma_start(mk, ap["mask"][b0:b0 + BT, :])

            # ---- per-object joint matmul + logit ----
            for k in range(K):
                if k + 1 < K:
                    # transposes for the next object ahead of this one's
                    # matmuls so the PE never waits on the copy engines
                    vts[k + 1] = load_vT(b0, k + 1)
                if k == 2 and bt + 1 < n_bt:
                    # prefetch the next b-tile's q transposes mid-loop
                    qt_cur = q_phase((bt + 1) * BT)
                vt = vts.pop(k)
                jp = jpsum.tile([128, NHID], F32, tag="j")
                for c in range(NV):
                    for j in range(NB):
                        nc.tensor.matmul(
                            jp[:, j * 512:(j + 1) * 512],
                            lhsT=vt[:, c, :],
                            rhs=w1vt[c][:, j * 512:(j + 1) * 512],
                            start=(c == 0),
                            stop=(c == NV - 1),
                        )
                if k == 0:
                    hq_sb = q_matmul(qt_cur)
                pre = prep.tile([128, NHID], F32)
                nc.vector.tensor_add(pre[:], jp[:], hq_sb[:])
                # logit_k = sum_h(relu(pre) * Wl) in one fused DVE pass
                nc.vector.scalar_tensor_tensor(
                    jp[:], pre[:], 0.0, wl[:],
                    op0=OP.max, op1=OP.mult,
                    accum_out=s6[:, 0, k:k + 1],
                )
            # + bl
            nc.vector.tensor_scalar_add(logits, logits, bl_ap)

            # ---- softmax over k ----
            nc.vector.tensor_reduce(s1[:, 0:1], logits, axis=AX.X, op=OP.max,
                                    negate=True)
            e_sm = s6[:, 2, :]
            nc.scalar.activation(e_sm, logits, AF.Exp, bias=s1[:, 0:1])
            nc.vector.tensor_reduce(s1[:, 1:2], e_sm, axis=AX.X, op=OP.add)
            nc.vector.reciprocal(s1[:, 2:3], s1[:, 1:2])
            att = s6[:, 3, :]
            nc.vector.tensor_scalar_mul(att, e_sm, s1[:, 2:3])
            nc.scalar.dma_start(ap["att"][b0:b0 + BT, :], att)

            # ---- context-erased scalar chain ----
            w_ctx = s6[:, 4, :]
            nc.vector.tensor_mul(w_ctx, logits, mk)
            nc.vector.tensor_reduce(s1[:, 3:4], w_ctx, axis=AX.X, op=OP.add)
            x = s6[:, 5, :]
            # x = (w_ctx - rowsum) * -1
            nc.vector.tensor_scalar(x, w_ctx, s1[:, 3:4], -1.0,
                                    op0=OP.subtract, op1=OP.mult)
            t1 = s6[:, 6, :]
            # t1 = relu(x) * c1
            nc.vector.tensor_scalar(t1, x, 0.0, c1_ap, op0=OP.max, op1=OP.mult)
            inner = s6[:, 7, :]
            # inner = x * c2 + t1
            nc.vector.scalar_tensor_tensor(inner, x, c2_ap, t1,
                                           op0=OP.mult, op1=OP.add)
            e_se = s6[:, 8, :]
            nc.scalar.activation(e_se, inner, AF.Exp, scale=-1.0)
            d = s6[:, 9, :]
            nc.vector.tensor_scalar_add(d, e_se, 1.0)
            r = s6[:, 10, :]
            nc.vector.reciprocal(r, d)
            g1 = s6[:, 11, :]
            nc.vector.tensor_mul(g1, x, e_se)
            gated = s6[:, 12, :]
            nc.vector.tensor_mul(gated, g1, r)
            s = s6[:, 13, :]
            nc.vector.tensor_add(s, gated, logits)

            # ---- ctx output: s * Wf + bf in one fused DVE pass per k;
            # on the final b-tile (nothing left to overlap) odd k's run on
            # ACT+GpSimd in parallel with DVE to shorten the serial tail ----
            for k in range(K):
                o = outp.tile([128, VDIM], F32)
                if bt == n_bt - 1 and k % 2 == 1:
                    nc.scalar.activation(o[:], wf[:], AF.Copy,
                                         scale=s6[:, 13, k:k + 1])
                    nc.gpsimd.tensor_tensor(out=o[:], in0=o[:], in1=bfb[:],
                                            op=OP.add)
                else:
                    nc.vector.scalar_tensor_tensor(o[:], wf[:],
                                                   s6[:, 13, k:k + 1],
                                                   bfb[:], op0=OP.mult,
                                                   op1=OP.add)
                nc.scalar.dma_start(ctx3[b0:b0 + BT, k, :], o[:])


def split_waits(nc, max_waits=1):
    """Walrus codegen rejects >1 sync wait on one instruction; split extras
    into preceding NoOps on the same engine."""
    for f in nc.m.functions:
        for bb in f.blocks:
            new_list = []
            changed = False
            for inst in bb.instructions:
                si = inst.sync_info
                if si is not None and len(si.on_wait) > max_waits:
                    waits = list(si.on_wait)
                    k = 0
                    while len(waits) > max_waits:
                        chunk, waits = waits[:max_waits], waits[max_waits:]
                        n = mybir.InstNoOp(name=f"{inst.name}-wsplit{k}",
                                           ins=[], outs=[])
                        n.engine = inst.engine
                        n.sync_info = mybir.SyncInfo(on_wait=chunk, on_update=[])
                        new_list.append(n)
                        k += 1
                    inst.sync_info = mybir.SyncInfo(on_wait=waits,
                                                    on_update=list(si.on_update))
                    changed = True
                new_list.append(inst)
            if changed:
                bb.instructions = new_list


def build(b_loc=B_LOC):
    nc = bass.Bass("TRN2", target_bir_lowering=False, debug=False)
    d = {}
    def din(name, shape, dt=F32):
        d[name] = nc.dram_tensor(name, shape, dt, kind="ExternalInput").ap()
    din("v", [b_loc, K, VDIM], F32R)
    din("q", [b_loc, QDIM], F32R)
    din("mask", [b_loc, K])
    din("w1vt", [NV, 128, NHID], F32R)
    din("w1qtb", [NQ + 1, 128, NHID], F32R)
    din("wl_b", [128, NHID])
    din("wf_b", [128, VDIM])
    din("bf_b", [128, VDIM])
    din("ident", [128, 128], F32R)
    din("ones_row", [1, 128], F32R)
    din("scal", [128, 8])
    d["att"] = nc.dram_tensor("att", [b_loc, K], F32, kind="ExternalOutput").ap()
    d["ctx"] = nc.dram_tensor("ctx", [b_loc * K, VDIM], F32,
                              kind="ExternalOutput").ap()
    with tile.TileContext(nc) as tc:
        emit(tc, d, b_loc)
    return nc


def host_consts(W1, b1, Wl, bl, Wse1, Wse2, Wf, bf):
    W1 = np.asarray(W1, np.float32)
    w1vt = np.ascontiguousarray(W1[:, :VDIM].T).reshape(NV, 128, NHID)
    w1qtb = np.zeros((NQ + 1, 128, NHID), np.float32)
    w1qtb[:NQ] = np.ascontiguousarray(W1[:, VDIM:].T).reshape(NQ, 128, NHID)
    w1qtb[NQ, 0, :] = np.asarray(b1, np.float32)
    wl_b = np.tile(np.asarray(Wl, np.float32).reshape(1, NHID), (128, 1))
    wf_b = np.tile(np.asarray(Wf, np.float32).reshape(1, VDIM), (128, 1))
    bf_b = np.tile(np.asarray(bf, np.float32).reshape(1, VDIM), (128, 1))
    a = np.asarray(Wse1, np.float64).reshape(-1)
    c = np.asarray(Wse2, np.float64).reshape(-1)
    P = float(np.sum(np.where(a > 0, a * c, 0.0)))
    Nn = float(np.sum(np.where(a < 0, a * c, 0.0)))
    scal = np.zeros((128, 8), np.float32)
    scal[:, 0] = P - Nn          # c1
    scal[:, 1] = Nn              # c2
    scal[:, 2] = float(np.asarray(bl).reshape(-1)[0])
    return {
        "w1vt": np.ascontiguousarray(w1vt),
        "w1qtb": w1qtb,
        "wl_b": np.ascontiguousarray(wl_b),
        "wf_b": np.ascontiguousarray(wf_b),
        "bf_b": np.ascontiguousarray(bf_b),
        "ident": np.eye(128, dtype=np.float32),
        "ones_row": np.ones((1, 128), np.float32),
        "scal": scal,
    }


_NC_CACHE = {}


def _get_nc(b_loc=B_LOC):
    if b_loc not in _NC_CACHE:
        nc = build(b_loc)
        split_waits(nc)
        _NC_CACHE[b_loc] = nc
    return _NC_CACHE[b_loc]


def run(inputs, trace=False):
    from concourse.bass_utils import run_bass_kernel_spmd

    v = np.ascontiguousarray(np.asarray(inputs["v"], np.float32))
    q = np.ascontiguousarray(np.asarray(inputs["q"], np.float32))
    mask = np.ascontiguousarray(np.asarray(inputs["mask"], np.float32))
    consts = host_consts(inputs["W1"], inputs["b1"], inputs["Wl"], inputs["bl"],
                         inputs["Wse1"], inputs["Wse2"], inputs["Wf"],
                         inputs["bf"])
    nc = _get_nc()
    in_maps = []
    for i in range(N_CORES):
        s = slice(i * B_LOC, (i + 1) * B_LOC)
        in_maps.append({
            "v": v[s], "q": q[s], "mask": mask[s], **consts,
        })
    res = run_bass_kernel_spmd(nc, in_maps, core_ids=list(range(N_CORES)),
                               trace=trace)
    att = np.concatenate([r["att"] for r in res.results], axis=0)
    ctxo = np.concatenate([r["ctx"] for r in res.results], axis=0)
    single_att = att.reshape(B, K, 1)
    context_erased_att = ctxo.reshape(B * K, 1, VDIM)
    return (single_att, context_erased_att), res


def kernel(**inputs):
    out, _ = run(inputs, trace=False)
    return out
